# revision 13
# baseline (speedup 1.0000x reference)
"""Batched per-adapter LoRA matmul on 8 TRN2 NeuronCores.

Problem: x [8, 4096, 2048] f32, weight [8, 2048, 64] f32
         out[b] = x[b] @ weight[b]    -> [8, 4096, 64] f32

Sharding: one batch/adapter per NeuronCore (S-LoRA style expert/data
parallelism). Per core: [4096, 2048] @ [2048, 64].

Device kernel computes out^T = w^T @ x^T: w chunks [128, 64] are the
stationary operand, x^T streams as the moving operand with the
contraction dim D on SBUF partitions. The host packs x into the exact
SBUF tile layout (so every DMA is one fully-contiguous block) and
unpacks the block-major output; both are pure layout work off the
critical path.

PE utilization: M=64 only fills half the 128-wide array, so two
s-blocks are computed concurrently via col-tiling — s-block A on PE
columns 0-63 (PSUM partitions 0-63), s-block B on columns 64-127
(PSUM partitions 64-127) with tile_position=(0, 64). Their
LDWEIGHTS/MATMUL chains overlap in hardware (per-subarray concurrency).

Matmul dtype modes (MODE below):
  f32    - plain fp32 (4 cyc/row on PE)
  bf16   - x and w rounded to bf16, single pass
  bf16w2 - x bf16, w split hi+lo bf16, 2 passes
  bf16x3 - x and w split hi+lo bf16, 3 passes (~4e-6 rel err)
"""

import numpy as np
import ml_dtypes

B, S, D, R = 8, 4096, 2048, 64
N_CORES = 8
P = 128
KO = D // P  # 16 contraction chunks of 128
SB = 512  # s-block (moving free dim / half-PSUM-bank chunk)
NSB = S // SB  # 8 s-blocks
SP = 2 * SB  # s-pair: two s-blocks computed concurrently
NSP = S // SP  # 4 s-pairs

MODE = "bf16"

BF16 = ml_dtypes.bfloat16

XBUFS = 3  # x tiles in flight (3 x 4 MB)
PSUM_BUFS = 2
OBUFS = 2
SPLIT_FIRST = True  # split pair-0 load into 2 MB halves so PE starts sooner


def _mode_config(mode):
    if mode == "f32":
        return ["xt"], ["w"], [("w", "xt")], np.float32
    if mode == "bf16":
        return ["xt"], ["w"], [("w", "xt")], BF16
    if mode == "bf16w2":
        return ["xt"], ["wh", "wl"], [("wh", "xt"), ("wl", "xt")], BF16
    if mode == "bf16x3":
        return (
            ["xh", "xl"],
            ["wh", "wl"],
            [("wh", "xh"), ("wh", "xl"), ("wl", "xh")],
            BF16,
        )
    raise ValueError(mode)


def _build_nc(mode):
    from concourse import bacc
    import concourse.mybir as mybir
    import concourse.tile as tile

    x_names, w_names, passes, np_dt = _mode_config(mode)
    dt = mybir.dt.float32 if np_dt is np.float32 else mybir.dt.bfloat16

    nc = bacc.Bacc(None, target_bir_lowering=False)

    # x packed host-side as [NSB, P, KO, SB]; flatten outer for 2D param.
    x_params = {
        n: nc.declare_dram_parameter(n, [NSB * P, KO * SB], dt, isOutput=False)
        for n in x_names
    }
    # w packed host-side as [P, KO, R].
    w_params = {
        n: nc.declare_dram_parameter(n, [P, KO * R], dt, isOutput=False)
        for n in w_names
    }
    # out blocks: [NSP, P, SB] where row p of pair q = s-block (2q + p//64),
    # r = p % 64. Host unpacks.
    out_param = nc.declare_dram_parameter(
        "ob", [NSP * P, SB], mybir.dt.float32, isOutput=True
    )

    with tile.TileContext(nc) as tc:
        with (
            tc.tile_pool(name="wpool", bufs=1) as wpool,
            tc.tile_pool(name="xpool", bufs=XBUFS) as xpool,
            tc.tile_pool(name="opool", bufs=OBUFS) as opool,
            tc.tile_pool(name="psum", bufs=PSUM_BUFS, space="PSUM") as psum_pool,
        ):
            w_tiles = {}
            for n in w_names:
                wt = wpool.tile([P, KO, R], dt, name=f"w_{n}")
                nc.sync.dma_start(wt[:], w_params[n].rearrange("p (ko r) -> p ko r", ko=KO))
                w_tiles[n] = wt

            n_mm = KO * len(passes)
            for q in range(NSP):
                # One 4 MB DMA covers s-blocks 2q (half 0) and 2q+1 (half 1).
                x_tiles = {}
                for n in x_names:
                    xt = xpool.tile([P, 2, KO, SB], dt, name=f"x_{n}", tag=f"x_{n}")
                    src = (
                        x_params[n]
                        .rearrange("(s p) f -> s p f", p=P)[2 * q : 2 * q + 2]
                        .rearrange("two p (ko f) -> p two ko f", ko=KO)
                    )
                    if q == 0 and SPLIT_FIRST:
                        # Two 2 MB DMAs so the A-chain can start sooner.
                        nc.sync.dma_start(xt[:, 0], src[:, 0])
                        nc.sync.dma_start(xt[:, 1], src[:, 1])
                    else:
                        nc.sync.dma_start(xt[:], src)
                    x_tiles[n] = xt

                psum = psum_pool.tile([P, SB], mybir.dt.float32, name="ps")
                i = 0
                for ko in range(KO):
                    for (wn, xn) in passes:
                        # s-block A on PE cols 0-63 -> PSUM partitions 0-63
                        nc.tensor.matmul(
                            psum[0:R, :],
                            lhsT=w_tiles[wn][:, ko, :],
                            rhs=x_tiles[xn][:, 0, ko, :],
                            start=(i == 0),
                            stop=(i == n_mm - 1),
                            tile_position=(0, 0),
                        )
                        # s-block B on PE cols 64-127 -> PSUM partitions 64-127
                        nc.tensor.matmul(
                            psum[R : 2 * R, :],
                            lhsT=w_tiles[wn][:, ko, :],
                            rhs=x_tiles[xn][:, 1, ko, :],
                            start=(i == 0),
                            stop=(i == n_mm - 1),
                            tile_position=(0, R),
                        )
                        i += 1

                o_tile = opool.tile([P, SB], mybir.dt.float32, name="o")
                nc.vector.tensor_copy(out=o_tile[:], in_=psum[:])
                # Stores go on the scalar HWDGE ring so they never queue
                # behind prefetched x loads on the sync ring.
                nc.scalar.dma_start(out_param[q * P : (q + 1) * P, :], o_tile[:])

    nc.finalize()
    return nc


_nc_cache = {}


def _get_nc(mode):
    if mode not in _nc_cache:
        _nc_cache[mode] = _build_nc(mode)
    return _nc_cache[mode]


def _pack_x(xb, np_dt):
    """[S, D] -> [NSB*P, KO*SB] tile-layout contiguous, cast to np_dt.

    arr[s*P + p, ko*SB + j] = xb[s*SB + j, ko*P + p]
    """
    a = xb.astype(np_dt) if np_dt is not np.float32 else xb
    # [S, D] -> [NSB, SB, KO, P] -> transpose to [NSB, P, KO, SB]
    a = a.reshape(NSB, SB, KO, P).transpose(0, 3, 2, 1)
    return np.ascontiguousarray(a).reshape(NSB * P, KO * SB)


def _pack_w(wb, np_dt):
    """[D, R] -> [P, KO*R]: arr[p, ko*R + r] = wb[ko*P + p, r]."""
    a = wb.astype(np_dt) if np_dt is not np.float32 else wb
    a = a.reshape(KO, P, R).transpose(1, 0, 2)
    return np.ascontiguousarray(a).reshape(P, KO * R)


def _unpack_out(ob):
    """[NSP*P, SB] -> [S, R] fp32.

    ob[q*P + p, j] = out[q*SP + (p//R)*SB + j, p%R]
    """
    a = ob.reshape(NSP, 2, R, SB)  # [q, half, r, j]
    a = a.transpose(0, 1, 3, 2)  # [q, half, j, r]
    return np.ascontiguousarray(a).reshape(S, R)


def _prep_inputs(x, weight, mode):
    x_names, w_names, passes, np_dt = _mode_config(mode)
    in_maps = []
    for b in range(B):
        m = {}
        if mode in ("f32", "bf16"):
            m["xt"] = _pack_x(x[b], np_dt)
            m["w"] = _pack_w(weight[b], np_dt)
        elif mode == "bf16w2":
            m["xt"] = _pack_x(x[b], BF16)
            wh = weight[b].astype(BF16)
            m["wh"] = _pack_w(wh, BF16)
            m["wl"] = _pack_w(
                (weight[b] - wh.astype(np.float32)), BF16
            )
        elif mode == "bf16x3":
            xh = x[b].astype(BF16)
            m["xh"] = _pack_x(xh, BF16)
            m["xl"] = _pack_x((x[b] - xh.astype(np.float32)), BF16)
            wh = weight[b].astype(BF16)
            m["wh"] = _pack_w(wh, BF16)
            m["wl"] = _pack_w((weight[b] - wh.astype(np.float32)), BF16)
        in_maps.append(m)
    return in_maps


def kernel(x, weight, mode=None, trace=False, _collect=None):
    """Full inputs in, full output out. Internally: 8-way batch-parallel."""
    from concourse import bass_utils

    mode = mode or MODE
    x = np.asarray(x, dtype=np.float32)
    weight = np.asarray(weight, dtype=np.float32)
    nc = _get_nc(mode)
    in_maps = _prep_inputs(x, weight, mode)
    res = bass_utils.run_bass_kernel_spmd(
        nc, in_maps, core_ids=list(range(N_CORES)), trace=trace
    )
    if _collect is not None:
        _collect.append(res)
    out = np.empty((B, S, R), dtype=np.float32)
    for b in range(B):
        out[b] = _unpack_out(res.results[b]["ob"])
    return out


# revision 15
# speedup vs baseline: 1.8965x; 1.8965x over previous
"""Batched per-adapter LoRA matmul on 8 TRN2 NeuronCores.

Problem: x [8, 4096, 2048] f32, weight [8, 2048, 64] f32
         out[b] = x[b] @ weight[b]    -> [8, 4096, 64] f32

Sharding: one batch/adapter per NeuronCore (S-LoRA style expert/data
parallelism). Per core: [4096, 2048] @ [2048, 64].

Device kernel computes out^T = w^T @ x^T: w chunks [128, 64] are the
stationary operand, x^T streams as the moving operand with the
contraction dim D on SBUF partitions. The host packs x into the exact
SBUF tile layout (so every DMA is one fully-contiguous block) and
unpacks the block-major output; both are pure layout work off the
critical path.

PE utilization: M=64 only fills half the 128-wide array, so two
s-blocks are computed concurrently via col-tiling — s-block A on PE
columns 0-63 (PSUM partitions 0-63), s-block B on columns 64-127
(PSUM partitions 64-127) with tile_position=(0, 64). Their
LDWEIGHTS/MATMUL chains overlap in hardware (per-subarray concurrency).

Matmul dtype modes (MODE below):
  f32    - plain fp32 (4 cyc/row on PE)
  bf16   - x and w rounded to bf16, single pass
  bf16w2 - x bf16, w split hi+lo bf16, 2 passes
  bf16x3 - x and w split hi+lo bf16, 3 passes (~4e-6 rel err)
"""

import numpy as np
import ml_dtypes

B, S, D, R = 8, 4096, 2048, 64
N_CORES = 8
P = 128
KO = D // P  # 16 contraction chunks of 128
SB = 512  # s-block (moving free dim / half-PSUM-bank chunk)
NSB = S // SB  # 8 s-blocks
SP = 2 * SB  # s-pair: two s-blocks computed concurrently
NSP = S // SP  # 4 s-pairs

MODE = "bf16"

BF16 = ml_dtypes.bfloat16

XBUFS = 3  # x tiles in flight (3 x 4 MB)
PSUM_BUFS = 2
OBUFS = 2
SPLIT_FIRST = True  # split pair-0 load into 2 MB halves so PE starts sooner


def _mode_config(mode):
    if mode == "f32":
        return ["xt"], ["w"], [("w", "xt")], np.float32
    if mode == "bf16":
        return ["xt"], ["w"], [("w", "xt")], BF16
    if mode == "bf16w2":
        return ["xt"], ["wh", "wl"], [("wh", "xt"), ("wl", "xt")], BF16
    if mode == "bf16x3":
        return (
            ["xh", "xl"],
            ["wh", "wl"],
            [("wh", "xh"), ("wh", "xl"), ("wl", "xh")],
            BF16,
        )
    raise ValueError(mode)


def _build_nc(mode):
    from concourse import bacc
    import concourse.mybir as mybir
    import concourse.tile as tile

    x_names, w_names, passes, np_dt = _mode_config(mode)
    dt = mybir.dt.float32 if np_dt is np.float32 else mybir.dt.bfloat16

    nc = bacc.Bacc(None, target_bir_lowering=False)

    # x packed host-side as [NSB, P, KO, SB]; flatten outer for 2D param.
    x_params = {
        n: nc.declare_dram_parameter(n, [NSB * P, KO * SB], dt, isOutput=False)
        for n in x_names
    }
    # w packed host-side as [P, KO, R].
    w_params = {
        n: nc.declare_dram_parameter(n, [P, KO * R], dt, isOutput=False)
        for n in w_names
    }
    # out blocks: [NSP, P, SB] where row p of pair q = s-block (2q + p//64),
    # r = p % 64. Host unpacks.
    out_param = nc.declare_dram_parameter(
        "ob", [NSP * P, SB], mybir.dt.float32, isOutput=True
    )

    with tile.TileContext(nc) as tc:
        with (
            tc.tile_pool(name="wpool", bufs=1) as wpool,
            tc.tile_pool(name="xpool", bufs=XBUFS) as xpool,
            tc.tile_pool(name="opool", bufs=OBUFS) as opool,
            tc.tile_pool(name="psum", bufs=PSUM_BUFS, space="PSUM") as psum_pool,
        ):
            w_tiles = {}
            for n in w_names:
                wt = wpool.tile([P, KO, R], dt, name=f"w_{n}")
                nc.sync.dma_start(wt[:], w_params[n].rearrange("p (ko r) -> p ko r", ko=KO))
                w_tiles[n] = wt

            n_mm = KO * len(passes)
            for q in range(NSP):
                # One 4 MB DMA covers s-blocks 2q (half 0) and 2q+1 (half 1).
                x_tiles = {}
                for n in x_names:
                    xt = xpool.tile([P, 2, KO, SB], dt, name=f"x_{n}", tag=f"x_{n}")
                    src = (
                        x_params[n]
                        .rearrange("(s p) f -> s p f", p=P)[2 * q : 2 * q + 2]
                        .rearrange("two p (ko f) -> p two ko f", ko=KO)
                    )
                    if q == 0 and SPLIT_FIRST:
                        # Two 2 MB DMAs so the A-chain can start sooner.
                        nc.sync.dma_start(xt[:, 0], src[:, 0])
                        nc.sync.dma_start(xt[:, 1], src[:, 1])
                    else:
                        nc.sync.dma_start(xt[:], src)
                    x_tiles[n] = xt

                psum = psum_pool.tile([P, SB], mybir.dt.float32, name="ps")
                i = 0
                for ko in range(KO):
                    for (wn, xn) in passes:
                        # s-block A on PE cols 0-63 -> PSUM partitions 0-63
                        nc.tensor.matmul(
                            psum[0:R, :],
                            lhsT=w_tiles[wn][:, ko, :],
                            rhs=x_tiles[xn][:, 0, ko, :],
                            start=(i == 0),
                            stop=(i == n_mm - 1),
                            tile_position=(0, 0),
                        )
                        # s-block B on PE cols 64-127 -> PSUM partitions 64-127
                        nc.tensor.matmul(
                            psum[R : 2 * R, :],
                            lhsT=w_tiles[wn][:, ko, :],
                            rhs=x_tiles[xn][:, 1, ko, :],
                            start=(i == 0),
                            stop=(i == n_mm - 1),
                            tile_position=(0, R),
                        )
                        i += 1

                o_tile = opool.tile([P, SB], mybir.dt.float32, name="o")
                nc.vector.tensor_copy(out=o_tile[:], in_=psum[:])
                # Stores go on the scalar HWDGE ring so they never queue
                # behind prefetched x loads on the sync ring.
                nc.scalar.dma_start(out_param[q * P : (q + 1) * P, :], o_tile[:])

    nc.finalize()
    return nc


_nc_cache = {}


def _get_nc(mode):
    if mode not in _nc_cache:
        _nc_cache[mode] = _build_nc(mode)
    return _nc_cache[mode]


def _pack_x(xall, np_dt):
    """[B, S, D] -> [B, NSB*P, KO*SB] tile-layout contiguous, cast to np_dt.

    arr[b, s*P + p, ko*SB + j] = xall[b, s*SB + j, ko*P + p]
    """
    a = xall.astype(np_dt) if np_dt is not np.float32 else xall
    a = a.reshape(B, NSB, SB, KO, P).transpose(0, 1, 4, 3, 2)
    return np.ascontiguousarray(a).reshape(B, NSB * P, KO * SB)


def _pack_w(wall, np_dt):
    """[B, D, R] -> [B, P, KO*R]: arr[b, p, ko*R + r] = wall[b, ko*P + p, r]."""
    a = wall.astype(np_dt) if np_dt is not np.float32 else wall
    a = a.reshape(B, KO, P, R).transpose(0, 2, 1, 3)
    return np.ascontiguousarray(a).reshape(B, P, KO * R)


def _unpack_out(ob):
    """[NSP*P, SB] -> [S, R] fp32.

    ob[q*P + p, j] = out[q*SP + (p//R)*SB + j, p%R]
    """
    a = ob.reshape(NSP, 2, R, SB)  # [q, half, r, j]
    a = a.transpose(0, 1, 3, 2)  # [q, half, j, r]
    return np.ascontiguousarray(a).reshape(S, R)


def _prep_inputs(x, weight, mode):
    _x_names, _w_names, _passes, np_dt = _mode_config(mode)
    bufs = {}
    if mode in ("f32", "bf16"):
        bufs["xt"] = _pack_x(x, np_dt)
        bufs["w"] = _pack_w(weight, np_dt)
    elif mode == "bf16w2":
        bufs["xt"] = _pack_x(x, BF16)
        wh = weight.astype(BF16)
        bufs["wh"] = _pack_w(wh, BF16)
        bufs["wl"] = _pack_w(weight - wh.astype(np.float32), BF16)
    elif mode == "bf16x3":
        xh = x.astype(BF16)
        bufs["xh"] = _pack_x(xh, BF16)
        bufs["xl"] = _pack_x(x - xh.astype(np.float32), BF16)
        wh = weight.astype(BF16)
        bufs["wh"] = _pack_w(wh, BF16)
        bufs["wl"] = _pack_w(weight - wh.astype(np.float32), BF16)
    return [{k: v[b] for k, v in bufs.items()} for b in range(B)]


def kernel(x, weight, mode=None, trace=False, _collect=None):
    """Full inputs in, full output out. Internally: 8-way batch-parallel."""
    from concourse import bass_utils

    mode = mode or MODE
    x = np.asarray(x, dtype=np.float32)
    weight = np.asarray(weight, dtype=np.float32)
    nc = _get_nc(mode)
    in_maps = _prep_inputs(x, weight, mode)
    res = bass_utils.run_bass_kernel_spmd(
        nc, in_maps, core_ids=list(range(N_CORES)), trace=trace
    )
    if _collect is not None:
        _collect.append(res)
    out = np.empty((B, S, R), dtype=np.float32)
    for b in range(B):
        out[b] = _unpack_out(res.results[b]["ob"])
    return out


# revision 19
# speedup vs baseline: 1.9974x; 1.0532x over previous
"""Batched per-adapter LoRA matmul on 8 TRN2 NeuronCores.

Problem: x [8, 4096, 2048] f32, weight [8, 2048, 64] f32
         out[b] = x[b] @ weight[b]    -> [8, 4096, 64] f32

Sharding: one batch/adapter per NeuronCore (S-LoRA style expert/data
parallelism). Per core: [4096, 2048] @ [2048, 64].

Device kernel computes out^T = w^T @ x^T: w chunks [128, 64] are the
stationary operand, x^T streams as the moving operand with the
contraction dim D on SBUF partitions. The host packs x into the exact
SBUF tile layout (so every DMA is one fully-contiguous block) and
unpacks the block-major output; both are pure layout work off the
critical path.

PE utilization: M=64 only fills half the 128-wide array, so two
s-blocks are computed concurrently via col-tiling — s-block A on PE
columns 0-63 (PSUM partitions 0-63), s-block B on columns 64-127
(PSUM partitions 64-127) with tile_position=(0, 64). Their
LDWEIGHTS/MATMUL chains overlap in hardware (per-subarray concurrency).

Matmul dtype modes (MODE below):
  f32    - plain fp32 (4 cyc/row on PE)
  bf16   - x and w rounded to bf16, single pass
  bf16w2 - x bf16, w split hi+lo bf16, 2 passes
  bf16x3 - x and w split hi+lo bf16, 3 passes (~4e-6 rel err)
"""

import numpy as np
import ml_dtypes

B, S, D, R = 8, 4096, 2048, 64
N_CORES = 8
P = 128
KO = D // P  # 16 contraction chunks of 128
SB = 512  # s-block (moving free dim / half-PSUM-bank chunk)
NSB = S // SB  # 8 s-blocks
SP = 2 * SB  # s-pair: two s-blocks computed concurrently
NSP = S // SP  # 4 s-pairs

MODE = "bf16"

BF16 = ml_dtypes.bfloat16

XBUFS = 3  # x tiles in flight (3 x 4 MB)
PSUM_BUFS = 2
OBUFS = 2
# split pair-0 load so PE starts sooner: 0=off, 1=2MB halves, 4=1MB quarters
SPLIT_FIRST = 1


def _mode_config(mode):
    if mode == "f32":
        return ["xt"], ["w"], [("w", "xt")], np.float32
    if mode == "bf16":
        return ["xt"], ["w"], [("w", "xt")], BF16
    if mode == "bf16w2":
        return ["xt"], ["wh", "wl"], [("wh", "xt"), ("wl", "xt")], BF16
    if mode == "bf16x3":
        return (
            ["xh", "xl"],
            ["wh", "wl"],
            [("wh", "xh"), ("wh", "xl"), ("wl", "xh")],
            BF16,
        )
    raise ValueError(mode)


def _build_nc(mode):
    from concourse import bacc
    import concourse.mybir as mybir
    import concourse.tile as tile

    x_names, w_names, passes, np_dt = _mode_config(mode)
    dt = mybir.dt.float32 if np_dt is np.float32 else mybir.dt.bfloat16

    nc = bacc.Bacc(None, target_bir_lowering=False)

    # x packed host-side as [NSB, P, KO, SB]; flatten outer for 2D param.
    x_params = {
        n: nc.declare_dram_parameter(n, [NSB * P, KO * SB], dt, isOutput=False)
        for n in x_names
    }
    # w packed host-side as [P, KO, R].
    w_params = {
        n: nc.declare_dram_parameter(n, [P, KO * R], dt, isOutput=False)
        for n in w_names
    }
    # out blocks: [NSP, P, SB] where row p of pair q = s-block (2q + p//64),
    # r = p % 64. Host unpacks.
    out_param = nc.declare_dram_parameter(
        "ob", [NSP * P, SB], mybir.dt.float32, isOutput=True
    )

    with tile.TileContext(nc) as tc:
        with (
            tc.tile_pool(name="wpool", bufs=1) as wpool,
            tc.tile_pool(name="xpool", bufs=XBUFS) as xpool,
            tc.tile_pool(name="opool", bufs=OBUFS) as opool,
            tc.tile_pool(name="psum", bufs=PSUM_BUFS, space="PSUM") as psum_pool,
        ):
            w_tiles = {}
            for n in w_names:
                wt = wpool.tile([P, KO, R], dt, name=f"w_{n}")
                nc.sync.dma_start(wt[:], w_params[n].rearrange("p (ko r) -> p ko r", ko=KO))
                w_tiles[n] = wt

            n_mm = KO * len(passes)
            for q in range(NSP):
                # One 4 MB DMA covers s-blocks 2q (half 0) and 2q+1 (half 1).
                x_tiles = {}
                for n in x_names:
                    xt = xpool.tile([P, 2, KO, SB], dt, name=f"x_{n}", tag=f"x_{n}")
                    src = (
                        x_params[n]
                        .rearrange("(s p) f -> s p f", p=P)[2 * q : 2 * q + 2]
                        .rearrange("two p (ko f) -> p two ko f", ko=KO)
                    )
                    if q == 0 and SPLIT_FIRST == 4:
                        # Four 1 MB DMAs, A-lo / B-lo first, so the first
                        # matmuls of both chains can start soonest.
                        h = KO // 2
                        nc.sync.dma_start(xt[:, 0, :h], src[:, 0, :h])
                        nc.sync.dma_start(xt[:, 1, :h], src[:, 1, :h])
                        nc.sync.dma_start(xt[:, 0, h:], src[:, 0, h:])
                        nc.sync.dma_start(xt[:, 1, h:], src[:, 1, h:])
                    elif q == 0 and SPLIT_FIRST:
                        # Two 2 MB DMAs so the A-chain can start sooner.
                        nc.sync.dma_start(xt[:, 0], src[:, 0])
                        nc.sync.dma_start(xt[:, 1], src[:, 1])
                    else:
                        nc.sync.dma_start(xt[:], src)
                    x_tiles[n] = xt

                psum = psum_pool.tile([P, SB], mybir.dt.float32, name="ps")
                i = 0
                for ko in range(KO):
                    for (wn, xn) in passes:
                        # s-block A on PE cols 0-63 -> PSUM partitions 0-63
                        nc.tensor.matmul(
                            psum[0:R, :],
                            lhsT=w_tiles[wn][:, ko, :],
                            rhs=x_tiles[xn][:, 0, ko, :],
                            start=(i == 0),
                            stop=(i == n_mm - 1),
                            tile_position=(0, 0),
                        )
                        # s-block B on PE cols 64-127 -> PSUM partitions 64-127
                        nc.tensor.matmul(
                            psum[R : 2 * R, :],
                            lhsT=w_tiles[wn][:, ko, :],
                            rhs=x_tiles[xn][:, 1, ko, :],
                            start=(i == 0),
                            stop=(i == n_mm - 1),
                            tile_position=(0, R),
                        )
                        i += 1

                o_tile = opool.tile([P, SB], mybir.dt.float32, name="o")
                nc.vector.tensor_copy(out=o_tile[:], in_=psum[:])
                # Stores go on the scalar HWDGE ring so they never queue
                # behind prefetched x loads on the sync ring.
                nc.scalar.dma_start(out_param[q * P : (q + 1) * P, :], o_tile[:])

    nc.finalize()
    return nc


_nc_cache = {}


def _get_nc(mode):
    if mode not in _nc_cache:
        _nc_cache[mode] = _build_nc(mode)
    return _nc_cache[mode]


def _pack_x(xall, np_dt):
    """[B, S, D] -> [B, NSB*P, KO*SB] tile-layout contiguous, cast to np_dt.

    arr[b, s*P + p, ko*SB + j] = xall[b, s*SB + j, ko*P + p]
    """
    a = xall.astype(np_dt) if np_dt is not np.float32 else xall
    a = a.reshape(B, NSB, SB, KO, P).transpose(0, 1, 4, 3, 2)
    return np.ascontiguousarray(a).reshape(B, NSB * P, KO * SB)


def _pack_w(wall, np_dt):
    """[B, D, R] -> [B, P, KO*R]: arr[b, p, ko*R + r] = wall[b, ko*P + p, r]."""
    a = wall.astype(np_dt) if np_dt is not np.float32 else wall
    a = a.reshape(B, KO, P, R).transpose(0, 2, 1, 3)
    return np.ascontiguousarray(a).reshape(B, P, KO * R)


def _unpack_out(ob):
    """[NSP*P, SB] -> [S, R] fp32.

    ob[q*P + p, j] = out[q*SP + (p//R)*SB + j, p%R]
    """
    a = ob.reshape(NSP, 2, R, SB)  # [q, half, r, j]
    a = a.transpose(0, 1, 3, 2)  # [q, half, j, r]
    return np.ascontiguousarray(a).reshape(S, R)


def _prep_inputs(x, weight, mode):
    _x_names, _w_names, _passes, np_dt = _mode_config(mode)
    bufs = {}
    if mode in ("f32", "bf16"):
        bufs["xt"] = _pack_x(x, np_dt)
        bufs["w"] = _pack_w(weight, np_dt)
    elif mode == "bf16w2":
        bufs["xt"] = _pack_x(x, BF16)
        wh = weight.astype(BF16)
        bufs["wh"] = _pack_w(wh, BF16)
        bufs["wl"] = _pack_w(weight - wh.astype(np.float32), BF16)
    elif mode == "bf16x3":
        xh = x.astype(BF16)
        bufs["xh"] = _pack_x(xh, BF16)
        bufs["xl"] = _pack_x(x - xh.astype(np.float32), BF16)
        wh = weight.astype(BF16)
        bufs["wh"] = _pack_w(wh, BF16)
        bufs["wl"] = _pack_w(weight - wh.astype(np.float32), BF16)
    return [{k: v[b] for k, v in bufs.items()} for b in range(B)]


def kernel(x, weight, mode=None, trace=False, _collect=None):
    """Full inputs in, full output out. Internally: 8-way batch-parallel."""
    from concourse import bass_utils

    mode = mode or MODE
    x = np.asarray(x, dtype=np.float32)
    weight = np.asarray(weight, dtype=np.float32)
    nc = _get_nc(mode)
    in_maps = _prep_inputs(x, weight, mode)
    try:
        res = bass_utils.run_bass_kernel_spmd(
            nc, in_maps, core_ids=list(range(N_CORES)), trace=trace
        )
    except Exception:
        # One retry with a freshly built program, in case of a transient
        # compile-cache or device hiccup.
        _nc_cache.pop(mode, None)
        nc = _get_nc(mode)
        res = bass_utils.run_bass_kernel_spmd(
            nc, in_maps, core_ids=list(range(N_CORES)), trace=trace
        )
    if _collect is not None:
        _collect.append(res)
    out = np.empty((B, S, R), dtype=np.float32)
    for b in range(B):
        out[b] = _unpack_out(res.results[b]["ob"])
    return out
